# revision 27
# baseline (speedup 1.0000x reference)
"""Trainium2 Bass kernel for nn_MinLoss_12343736009330 (v2: fp8 DoubleRow).

Math: the reference loss is
    loss = sum_{b,s} || pf[b,s] - gf[b,match[b,s]] ||_2
where pf/gf are the per-(batch, source) flattened [L=T*D] signals, and match
is a greedy assignment on the 4x4 Euclidean cdist.  Since
    ||pf[s] - gf[m]||^2 = pn[s] + gn[m] - 2 <pf[s], gf[m]>,
the whole computation reduces to the per-batch 8x8 Gram matrix of the
8 vectors {pf[0..4], gf[0..4]} plus a tiny 4x4 greedy matching.

Key change vs v1: the host pre-casts the interleaved operand layout to
fp8 e4m3 (loss rel-err from input rounding ~4e-4, tolerance is 2e-2), which
cuts HBM traffic 4x vs fp32, and the Gram matmuls run in DoubleRow perf
mode (2 contraction rows per PE column-cycle).  DMA becomes the roofline:
4 MiB/core at ~332 GB/s ~= 12.6 us.

Sharding: batch axis (16) across 8 cores -> 2 batches/core.  Per batch the
t=512 contraction is covered by 2 tiles of [128 partitions, 2 (DoubleRow
halves), 4096 interleaved columns]; columns interleave (d-slice, vector) so
the 16 8x8 diagonal blocks of each accumulated 128x128 matmul hold per-
d-slice Gram contributions.  Selector matmuls reduce the diagonal blocks,
flatten matmuls put the Gram on one partition, and the greedy matching runs
on-device (min -> one-hot mask -> row/col conflict flags -> additive BIG
penalty).  The 8 greedy minima (squared) go back; host does sqrt + sum.
"""

import os
import sys

import ml_dtypes
import numpy as np

try:
    import concourse.bass as bass  # noqa: F401
except ImportError:
    sys.path.insert(0, "/opt/trn_rl_repo")

import concourse.bass as bass  # noqa: F811
import concourse.tile as tile
from concourse import bacc, mybir
from concourse.bass_utils import run_bass_kernel_spmd


def _install_ntff_hook_shim():
    """The bare agent image lacks ``antenv.axon_hooks``, so trace=True under
    axon would ImportError.  Recreate the module with the ctypes-based NTFF
    hook from trn_agent_boot (degrades to hook=None if unavailable)."""
    import types

    try:
        import antenv.axon_hooks  # noqa: F401

        return
    except ImportError:
        pass
    hook = None
    try:
        from trn_agent_boot.trn_boot import _ntff_profile_via_ctypes

        so_path = "/opt/axon/libaxon_pjrt.so"
        if os.path.exists(so_path):
            hook = _ntff_profile_via_ctypes(so_path)
    except Exception:
        hook = None
    import antenv

    mod = types.ModuleType("antenv.axon_hooks")
    mod.get_axon_ntff_profile_hook = lambda: hook  # type: ignore[attr-defined]

    def _set(h):
        nonlocal hook
        hook = h

    mod.set_axon_ntff_profile_hook = _set  # type: ignore[attr-defined]
    sys.modules["antenv.axon_hooks"] = mod
    antenv.axon_hooks = mod


_install_ntff_hook_shim()

F32 = mybir.dt.float32
BF16 = mybir.dt.bfloat16

S, T, B, D = 4, 512, 16, 512
N_CORES = 8
NB = B // N_CORES          # batches per core
NTBP = 2                   # t-block pairs per batch (4 blocks of 128 rows)
NV = 2 * S                 # 8 vectors per batch (4 preds + 4 gts)
NCOL = NV * D              # 4096 interleaved columns per DoubleRow half
NG = NCOL // 128           # 32 column groups per half
BIG = 1.0e30

# "fp8": e4m3 operands + DoubleRow matmuls (2 rows/cycle).
# "bf16": bf16 operands, plain matmuls.  Host pre-casts either way.
VARIANT = os.environ.get("MINLOSS_VARIANT", "fp8")
NSTRIP = int(os.environ.get("MINLOSS_NSTRIP", "4"))
# DMA issue queues, round-robin per strip: s=sync a=scalar v=vector g=gpsimd
QUEUES = os.environ.get("MINLOSS_QUEUES", "sag")

ID8 = np.eye(8, dtype=np.float32)
ID128 = np.eye(128, dtype=ml_dtypes.bfloat16)


def _bass_dt(variant: str):
    return mybir.dt.float8e4 if variant == "fp8" else BF16


def _np_dt(variant: str):
    return ml_dtypes.float8_e4m3 if variant == "fp8" else ml_dtypes.bfloat16


def build_nc(variant: str, nstrip: int, queues: str):
    nc = bacc.Bacc(
        "TRN2",
        target_bir_lowering=False,
        debug=False,
        enable_asserts=True,
        num_devices=N_CORES,
    )
    bdt = _bass_dt(variant)
    # xa[b, tbp, p, i, g*128 + ii*8 + v]: vector v's value at
    # t = 256*tbp + 128*i + p, d = 16*g + ii (v 0..3 preds, 4..7 gts).
    xa_t = nc.dram_tensor(
        "xa", [NB, NTBP, 128, 2, NCOL], bdt, kind="ExternalInput"
    ).ap()
    id8_t = nc.dram_tensor("id8", [8, 8], F32, kind="ExternalInput").ap()
    id128_t = nc.dram_tensor("id128", [128, 128], BF16, kind="ExternalInput").ap()
    # the 8 greedy minima (squared distances); host does sqrt + sum
    loss_t = nc.dram_tensor("loss", [1, 2 * S], F32, kind="ExternalOutput").ap()

    with tile.TileContext(nc) as tc:
        _build_tile(tc, xa_t, id8_t, id128_t, loss_t, variant, nstrip, queues)

    nc.compile()
    return nc


def _build_tile(tc, xa_t, id8_t, id128_t, loss_t, variant, nstrip, queues):
    nc = tc.nc
    import contextlib

    ctx = contextlib.ExitStack()
    with ctx:
        b_pool = ctx.enter_context(tc.tile_pool(name="b", bufs=NB * NTBP))
        psum_pool = ctx.enter_context(tc.tile_pool(name="psum", bufs=NB, space="PSUM"))
        psumf_pool = ctx.enter_context(tc.tile_pool(name="psumf", bufs=2, space="PSUM"))
        consts_pool = ctx.enter_context(tc.tile_pool(name="consts", bufs=1))
        small_pool = ctx.enter_context(tc.tile_pool(name="small", bufs=2))

        bdt = _bass_dt(variant)
        qmap = {"s": nc.sync, "a": nc.scalar, "v": nc.vector, "g": nc.gpsimd}
        qlist = [qmap[ch] for ch in queues]

        id8 = consts_pool.tile([8, 8], F32, tag="id8")
        idb = consts_pool.tile([128, 128], BF16, tag="idb")
        nc.gpsimd.dma_start(out=id8[:, :], in_=id8_t[:, :])
        nc.gpsimd.dma_start(out=idb[:, :], in_=id128_t[:, :])

        # the 8 greedy minima (squared dists); sqrt+sum on host at the end
        loss4 = small_pool.tile([1, 2 * S], F32, tag="loss4")

        # ======== phase 0: issue ALL input strips up front ========
        # Strips round-robin over the issue queues so the DMA engines see a
        # continuous descriptor supply; tiles land roughly in order, matmuls
        # chase the strips.
        tiles = []
        qi = 0
        cw = NCOL // nstrip
        for ib in range(NB):
            for tbp in range(NTBP):
                btl = b_pool.tile([128, 2, NCOL], bdt, name=f"btl_{ib}_{tbp}")
                tiles.append(btl)
                for st in range(nstrip):
                    sl = slice(st * cw, (st + 1) * cw)
                    q = qlist[qi % len(qlist)]
                    qi += 1
                    q.dma_start(out=btl[:, :, sl], in_=xa_t[ib, tbp, :, :, sl])

        # ======== per batch: Gram matmuls + reduction + matching ========
        for ib in range(NB):
            psum = psum_pool.tile([128, 128], F32, name=f"psum_{ib}")
            for tbp in range(NTBP):
                btl = tiles[ib * NTBP + tbp]
                for g in range(NG):
                    first = tbp == 0 and g == 0
                    last = tbp == NTBP - 1 and g == NG - 1
                    if variant == "fp8":
                        op = btl[:, :, g * 128 : (g + 1) * 128]
                        nc.tensor.matmul(
                            psum[:, :],
                            lhsT=op,
                            rhs=op,
                            start=first,
                            stop=last,
                            perf_mode=mybir.MatmulPerfMode.DoubleRow,
                        )
                    else:
                        for i in range(2):
                            opi = btl[:, i, g * 128 : (g + 1) * 128]
                            nc.tensor.matmul(
                                psum[:, :],
                                lhsT=opi,
                                rhs=opi,
                                start=first and i == 0,
                                stop=last and i == 1,
                            )

            # ---------------- diagonal-block reduction (on PE) ------------
            # Engine APs can't start at partition 8q, so selector matmuls
            # I128[:,8q:8q+8].T @ C[:,8q:8q+8] move block q to partitions
            # 0:8 and PSUM-accumulate over q.  bf16 copy halves DVE/ident
            # cost; partial-Gram bf16 rounding is ~1e-4 on the loss.
            c_sb = small_pool.tile([128, 128], BF16, name=f"c_sb_{ib}")
            nc.vector.tensor_copy(out=c_sb[:, :], in_=psum[:, :])
            psg = psumf_pool.tile([8, 8], F32, tag="psg", name=f"psg_{ib}")
            for q in range(16):
                nc.tensor.matmul(
                    psg[:, :],
                    lhsT=idb[:, 8 * q : 8 * q + 8],
                    rhs=c_sb[:, 8 * q : 8 * q + 8],
                    start=(q == 0),
                    stop=(q == 15),
                )
            acc = small_pool.tile([8, 8], F32, name=f"acc_{ib}")
            nc.vector.tensor_copy(out=acc[:, :], in_=psg[:, :])

            # ---------------- flatten Gram to one partition ----------------
            psf = psumf_pool.tile([1, 72], F32, tag="psf", name=f"psf_{ib}")
            for p in range(8):
                nc.tensor.matmul(
                    psf[0:1, 8 * p : 8 * p + 8],
                    lhsT=id8[:, p : p + 1],
                    rhs=acc[:, :],
                    start=True,
                    stop=True,
                )

            # ------------- d2 = pn + gn - 2*cross, read from PSUM --------
            # (sqrt is monotone, so the greedy matching runs on d2; the
            # sqrt of the 8 collected minima happens on the host)
            g9 = psf[0:1, 0:72].rearrange("p (a b) -> p a b", b=9)
            pn = g9[:, 0:4, 0:1].broadcast_to((1, 4, 4))
            gn = g9[:, 4:8, 0:1].transpose([0, 2, 1]).broadcast_to((1, 4, 4))
            cross = psf[0:1, 0:64].rearrange("p (a b) -> p a b", b=8)[:, 0:4, 4:8]

            d2 = small_pool.tile([1, 16], F32, name=f"d2_{ib}")
            d2v = d2[0:1, :].rearrange("p (a b) -> p a b", b=4)
            nc.vector.tensor_scalar(
                out=d2v,
                in0=cross,
                scalar1=-2.0,
                scalar2=None,
                op0=mybir.AluOpType.mult,
            )
            nc.vector.tensor_add(out=d2v, in0=d2v, in1=pn)
            nc.vector.tensor_add(out=d2v, in0=d2v, in1=gn)

            # ---------------- greedy matching on d2 ----------------
            # per iteration: min -> {0,BIG} mask of the argmin -> row/col
            # conflict flags (max-reduce over the 4x4 mask) -> add both
            # flags into d2.  (On an exact fp32 tie both tied entries are
            # masked; the resulting loss difference is O(tie gap).)
            mask16 = small_pool.tile([1, 16], F32, name=f"mask16_{ib}")
            rc = small_pool.tile([1, 8], F32, name=f"rc_{ib}")
            m44 = mask16[0:1, :].rearrange("p (r c) -> p r c", c=4)
            m44t = m44.transpose([0, 2, 1])
            rcv = rc[0:1, :].rearrange("p (x y) -> p x y", y=4)
            rowb = rcv[:, 0:1, :].transpose([0, 2, 1]).broadcast_to((1, 4, 4))
            colb = rcv[:, 1:2, :].broadcast_to((1, 4, 4))

            for it in range(S):
                slot = loss4[0:1, ib * S + it : ib * S + it + 1]
                nc.vector.tensor_reduce(
                    out=slot,
                    in_=d2[:, :],
                    axis=mybir.AxisListType.X,
                    op=mybir.AluOpType.min,
                )
                if it == S - 1:
                    break
                nc.vector.tensor_scalar(
                    out=mask16[:, :],
                    in0=d2[:, :],
                    scalar1=slot,
                    scalar2=BIG,
                    op0=mybir.AluOpType.is_le,
                    op1=mybir.AluOpType.mult,
                )
                nc.vector.tensor_reduce(
                    out=rc[0:1, 0:4],
                    in_=m44,
                    axis=mybir.AxisListType.X,
                    op=mybir.AluOpType.max,
                )
                nc.vector.tensor_reduce(
                    out=rc[0:1, 4:8],
                    in_=m44t,
                    axis=mybir.AxisListType.X,
                    op=mybir.AluOpType.max,
                )
                nc.vector.tensor_add(out=d2v, in0=d2v, in1=rowb)
                nc.vector.tensor_add(out=d2v, in0=d2v, in1=colb)

        nc.sync.dma_start(out=loss_t[0:1, :], in_=loss4[:, :])


_NC_CACHE: dict = {}


def _get_nc():
    key = (VARIANT, NSTRIP, QUEUES)
    if key not in _NC_CACHE:
        _NC_CACHE[key] = build_nc(*key)
    return _NC_CACHE[key]


def shard_inputs(preds: np.ndarray, gts: np.ndarray, variant: str):
    """Build the interleaved low-precision layout
    X[b, tbp, p, i, g*128 + ii*8 + v] and slice per core (b outermost, so
    per-core slices are contiguous views)."""
    npdt = _np_dt(variant)
    p8 = np.asarray(preds).astype(npdt)
    g8 = np.asarray(gts).astype(npdt)
    X = np.empty((B, NTBP, 128, 2, 32, 16, NV), npdt)
    # preds [S, T, B, D] -> [b, tbp, p, i, g, ii, s]
    X[..., 0:S] = p8.reshape(S, 2, 2, 128, B, 32, 16).transpose(4, 1, 3, 2, 5, 6, 0)
    # gts [S, B, T, D] -> [b, tbp, p, i, g, ii, s]
    X[..., S : 2 * S] = g8.reshape(S, B, 2, 2, 128, 32, 16).transpose(
        1, 2, 4, 3, 5, 6, 0
    )
    X = X.reshape(B, NTBP, 128, 2, NCOL)
    in_maps = []
    for c in range(N_CORES):
        b0 = c * NB
        in_maps.append({"xa": X[b0 : b0 + NB], "id8": ID8, "id128": ID128})
    return in_maps


kernel_last_results = None


def kernel(preds: np.ndarray, gts: np.ndarray) -> np.ndarray:
    global kernel_last_results
    nc = _get_nc()
    in_maps = shard_inputs(preds, gts, VARIANT)
    trace = os.environ.get("MINLOSS_TRACE", "1") == "1"
    try:
        res = run_bass_kernel_spmd(
            nc, in_maps, core_ids=list(range(N_CORES)), trace=trace
        )
    except Exception:
        if not trace:
            raise
        # profiling infrastructure may be unavailable; rerun without it
        res = run_bass_kernel_spmd(
            nc, in_maps, core_ids=list(range(N_CORES)), trace=False
        )
    kernel_last_results = res
    total = 0.0
    for c in range(N_CORES):
        m2 = np.asarray(res.results[c]["loss"], dtype=np.float64)
        total += float(np.sqrt(np.maximum(m2, 0.0)).sum())
    return np.array(total, dtype=np.float32)


# revision 28
# speedup vs baseline: 1.0730x; 1.0730x over previous
"""Trainium2 Bass kernel for nn_MinLoss_12343736009330 (v2: fp8 DoubleRow).

Math: the reference loss is
    loss = sum_{b,s} || pf[b,s] - gf[b,match[b,s]] ||_2
where pf/gf are the per-(batch, source) flattened [L=T*D] signals, and match
is a greedy assignment on the 4x4 Euclidean cdist.  Since
    ||pf[s] - gf[m]||^2 = pn[s] + gn[m] - 2 <pf[s], gf[m]>,
the whole computation reduces to the per-batch 8x8 Gram matrix of the
8 vectors {pf[0..4], gf[0..4]} plus a tiny 4x4 greedy matching.

Key change vs v1: the host pre-casts the interleaved operand layout to
fp8 e4m3 (loss rel-err from input rounding ~4e-4, tolerance is 2e-2), which
cuts HBM traffic 4x vs fp32, and the Gram matmuls run in DoubleRow perf
mode (2 contraction rows per PE column-cycle).  DMA becomes the roofline:
4 MiB/core at ~332 GB/s ~= 12.6 us.

Sharding: batch axis (16) across 8 cores -> 2 batches/core.  Per batch the
t=512 contraction is covered by 2 tiles of [128 partitions, 2 (DoubleRow
halves), 4096 interleaved columns]; columns interleave (d-slice, vector) so
the 16 8x8 diagonal blocks of each accumulated 128x128 matmul hold per-
d-slice Gram contributions.  Selector matmuls reduce the diagonal blocks,
flatten matmuls put the Gram on one partition, and the greedy matching runs
on-device (min -> one-hot mask -> row/col conflict flags -> additive BIG
penalty).  The 8 greedy minima (squared) go back; host does sqrt + sum.
"""

import os
import sys

import ml_dtypes
import numpy as np

try:
    import concourse.bass as bass  # noqa: F401
except ImportError:
    sys.path.insert(0, "/opt/trn_rl_repo")

import concourse.bass as bass  # noqa: F811
import concourse.tile as tile
from concourse import bacc, mybir
from concourse.bass_utils import run_bass_kernel_spmd


def _install_ntff_hook_shim():
    """The bare agent image lacks ``antenv.axon_hooks``, so trace=True under
    axon would ImportError.  Recreate the module with the ctypes-based NTFF
    hook from trn_agent_boot (degrades to hook=None if unavailable)."""
    import types

    try:
        import antenv.axon_hooks  # noqa: F401

        return
    except ImportError:
        pass
    hook = None
    try:
        from trn_agent_boot.trn_boot import _ntff_profile_via_ctypes

        so_path = "/opt/axon/libaxon_pjrt.so"
        if os.path.exists(so_path):
            hook = _ntff_profile_via_ctypes(so_path)
    except Exception:
        hook = None
    import antenv

    mod = types.ModuleType("antenv.axon_hooks")
    mod.get_axon_ntff_profile_hook = lambda: hook  # type: ignore[attr-defined]

    def _set(h):
        nonlocal hook
        hook = h

    mod.set_axon_ntff_profile_hook = _set  # type: ignore[attr-defined]
    sys.modules["antenv.axon_hooks"] = mod
    antenv.axon_hooks = mod


_install_ntff_hook_shim()

F32 = mybir.dt.float32
BF16 = mybir.dt.bfloat16

S, T, B, D = 4, 512, 16, 512
N_CORES = 8
NB = B // N_CORES          # batches per core
NTBP = 2                   # t-block pairs per batch (4 blocks of 128 rows)
NV = 2 * S                 # 8 vectors per batch (4 preds + 4 gts)
NCOL = NV * D              # 4096 interleaved columns per DoubleRow half
NG = NCOL // 128           # 32 column groups per half
BIG = 1.0e30

# "fp8": e4m3 operands + DoubleRow matmuls (2 rows/cycle).
# "bf16": bf16 operands, plain matmuls.  Host pre-casts either way.
VARIANT = os.environ.get("MINLOSS_VARIANT", "fp8")
NSTRIP = int(os.environ.get("MINLOSS_NSTRIP", "4"))
# DMA issue queues, round-robin per strip: s=sync a=scalar v=vector g=gpsimd
QUEUES = os.environ.get("MINLOSS_QUEUES", "sa")

ID8 = np.eye(8, dtype=np.float32)
ID128 = np.eye(128, dtype=ml_dtypes.bfloat16)


def _bass_dt(variant: str):
    return mybir.dt.float8e4 if variant == "fp8" else BF16


def _np_dt(variant: str):
    return ml_dtypes.float8_e4m3 if variant == "fp8" else ml_dtypes.bfloat16


def build_nc(variant: str, nstrip: int, queues: str):
    nc = bacc.Bacc(
        "TRN2",
        target_bir_lowering=False,
        debug=False,
        enable_asserts=True,
        num_devices=N_CORES,
    )
    bdt = _bass_dt(variant)
    # xa[b, tbp, p, i, g*128 + ii*8 + v]: vector v's value at
    # t = 256*tbp + 128*i + p, d = 16*g + ii (v 0..3 preds, 4..7 gts).
    xa_t = nc.dram_tensor(
        "xa", [NB, NTBP, 128, 2, NCOL], bdt, kind="ExternalInput"
    ).ap()
    id8_t = nc.dram_tensor("id8", [8, 8], F32, kind="ExternalInput").ap()
    id128_t = nc.dram_tensor("id128", [128, 128], BF16, kind="ExternalInput").ap()
    # the 8 greedy minima (squared distances); host does sqrt + sum
    loss_t = nc.dram_tensor("loss", [1, 2 * S], F32, kind="ExternalOutput").ap()

    with tile.TileContext(nc) as tc:
        _build_tile(tc, xa_t, id8_t, id128_t, loss_t, variant, nstrip, queues)

    nc.compile()
    return nc


def _build_tile(tc, xa_t, id8_t, id128_t, loss_t, variant, nstrip, queues):
    nc = tc.nc
    import contextlib

    ctx = contextlib.ExitStack()
    with ctx:
        b_pool = ctx.enter_context(tc.tile_pool(name="b", bufs=NB * NTBP))
        psum_pool = ctx.enter_context(tc.tile_pool(name="psum", bufs=NB, space="PSUM"))
        psumf_pool = ctx.enter_context(tc.tile_pool(name="psumf", bufs=2, space="PSUM"))
        consts_pool = ctx.enter_context(tc.tile_pool(name="consts", bufs=1))
        small_pool = ctx.enter_context(tc.tile_pool(name="small", bufs=2))

        bdt = _bass_dt(variant)
        qmap = {"s": nc.sync, "a": nc.scalar, "v": nc.vector, "g": nc.gpsimd}
        qlist = [qmap[ch] for ch in queues]

        id8 = consts_pool.tile([8, 8], F32, tag="id8")
        idb = consts_pool.tile([128, 128], BF16, tag="idb")
        nc.gpsimd.dma_start(out=id8[:, :], in_=id8_t[:, :])
        nc.gpsimd.dma_start(out=idb[:, :], in_=id128_t[:, :])

        # the 8 greedy minima (squared dists); sqrt+sum on host at the end
        loss4 = small_pool.tile([1, 2 * S], F32, tag="loss4")

        # ======== phase 0: issue ALL input strips up front ========
        # Strips round-robin over the issue queues so the DMA engines see a
        # continuous descriptor supply; tiles land roughly in order, matmuls
        # chase the strips.
        tiles = []
        qi = 0
        cw = NCOL // nstrip
        for ib in range(NB):
            for tbp in range(NTBP):
                btl = b_pool.tile([128, 2, NCOL], bdt, name=f"btl_{ib}_{tbp}")
                tiles.append(btl)
                for st in range(nstrip):
                    sl = slice(st * cw, (st + 1) * cw)
                    q = qlist[qi % len(qlist)]
                    qi += 1
                    q.dma_start(out=btl[:, :, sl], in_=xa_t[ib, tbp, :, :, sl])

        # ======== per batch: Gram matmuls + reduction + matching ========
        for ib in range(NB):
            psum = psum_pool.tile([128, 128], F32, name=f"psum_{ib}")
            for tbp in range(NTBP):
                btl = tiles[ib * NTBP + tbp]
                for g in range(NG):
                    first = tbp == 0 and g == 0
                    last = tbp == NTBP - 1 and g == NG - 1
                    if variant == "fp8":
                        op = btl[:, :, g * 128 : (g + 1) * 128]
                        nc.tensor.matmul(
                            psum[:, :],
                            lhsT=op,
                            rhs=op,
                            start=first,
                            stop=last,
                            perf_mode=mybir.MatmulPerfMode.DoubleRow,
                        )
                    else:
                        for i in range(2):
                            opi = btl[:, i, g * 128 : (g + 1) * 128]
                            nc.tensor.matmul(
                                psum[:, :],
                                lhsT=opi,
                                rhs=opi,
                                start=first and i == 0,
                                stop=last and i == 1,
                            )

            # ---------------- diagonal-block reduction (on PE) ------------
            # Engine APs can't start at partition 8q, so selector matmuls
            # I128[:,8q:8q+8].T @ C[:,8q:8q+8] move block q to partitions
            # 0:8 and PSUM-accumulate over q.  bf16 copy halves DVE/ident
            # cost; partial-Gram bf16 rounding is ~1e-4 on the loss.
            c_sb = small_pool.tile([128, 128], BF16, name=f"c_sb_{ib}")
            nc.vector.tensor_copy(out=c_sb[:, :], in_=psum[:, :])
            psg = psumf_pool.tile([8, 8], F32, tag="psg", name=f"psg_{ib}")
            for q in range(16):
                nc.tensor.matmul(
                    psg[:, :],
                    lhsT=idb[:, 8 * q : 8 * q + 8],
                    rhs=c_sb[:, 8 * q : 8 * q + 8],
                    start=(q == 0),
                    stop=(q == 15),
                )
            acc = small_pool.tile([8, 8], F32, name=f"acc_{ib}")
            nc.vector.tensor_copy(out=acc[:, :], in_=psg[:, :])

            # ---------------- flatten Gram to one partition ----------------
            psf = psumf_pool.tile([1, 72], F32, tag="psf", name=f"psf_{ib}")
            for p in range(8):
                nc.tensor.matmul(
                    psf[0:1, 8 * p : 8 * p + 8],
                    lhsT=id8[:, p : p + 1],
                    rhs=acc[:, :],
                    start=True,
                    stop=True,
                )

            # ------------- d2 = pn + gn - 2*cross, read from PSUM --------
            # (sqrt is monotone, so the greedy matching runs on d2; the
            # sqrt of the 8 collected minima happens on the host)
            g9 = psf[0:1, 0:72].rearrange("p (a b) -> p a b", b=9)
            pn = g9[:, 0:4, 0:1].broadcast_to((1, 4, 4))
            gn = g9[:, 4:8, 0:1].transpose([0, 2, 1]).broadcast_to((1, 4, 4))
            cross = psf[0:1, 0:64].rearrange("p (a b) -> p a b", b=8)[:, 0:4, 4:8]

            d2 = small_pool.tile([1, 16], F32, name=f"d2_{ib}")
            d2v = d2[0:1, :].rearrange("p (a b) -> p a b", b=4)
            nc.vector.tensor_scalar(
                out=d2v,
                in0=cross,
                scalar1=-2.0,
                scalar2=None,
                op0=mybir.AluOpType.mult,
            )
            nc.vector.tensor_add(out=d2v, in0=d2v, in1=pn)
            nc.vector.tensor_add(out=d2v, in0=d2v, in1=gn)

            # ---------------- greedy matching on d2 ----------------
            # per iteration: min -> {0,BIG} mask of the argmin -> row/col
            # conflict flags (max-reduce over the 4x4 mask) -> add both
            # flags into d2.  (On an exact fp32 tie both tied entries are
            # masked; the resulting loss difference is O(tie gap).)
            mask16 = small_pool.tile([1, 16], F32, name=f"mask16_{ib}")
            rc = small_pool.tile([1, 8], F32, name=f"rc_{ib}")
            m44 = mask16[0:1, :].rearrange("p (r c) -> p r c", c=4)
            m44t = m44.transpose([0, 2, 1])
            rcv = rc[0:1, :].rearrange("p (x y) -> p x y", y=4)
            rowb = rcv[:, 0:1, :].transpose([0, 2, 1]).broadcast_to((1, 4, 4))
            colb = rcv[:, 1:2, :].broadcast_to((1, 4, 4))

            for it in range(S):
                slot = loss4[0:1, ib * S + it : ib * S + it + 1]
                nc.vector.tensor_reduce(
                    out=slot,
                    in_=d2[:, :],
                    axis=mybir.AxisListType.X,
                    op=mybir.AluOpType.min,
                )
                if it == S - 1:
                    break
                nc.vector.tensor_scalar(
                    out=mask16[:, :],
                    in0=d2[:, :],
                    scalar1=slot,
                    scalar2=BIG,
                    op0=mybir.AluOpType.is_le,
                    op1=mybir.AluOpType.mult,
                )
                nc.vector.tensor_reduce(
                    out=rc[0:1, 0:4],
                    in_=m44,
                    axis=mybir.AxisListType.X,
                    op=mybir.AluOpType.max,
                )
                nc.vector.tensor_reduce(
                    out=rc[0:1, 4:8],
                    in_=m44t,
                    axis=mybir.AxisListType.X,
                    op=mybir.AluOpType.max,
                )
                nc.vector.tensor_add(out=d2v, in0=d2v, in1=rowb)
                nc.vector.tensor_add(out=d2v, in0=d2v, in1=colb)

        nc.sync.dma_start(out=loss_t[0:1, :], in_=loss4[:, :])


_NC_CACHE: dict = {}


def _get_nc():
    key = (VARIANT, NSTRIP, QUEUES)
    if key not in _NC_CACHE:
        _NC_CACHE[key] = build_nc(*key)
    return _NC_CACHE[key]


def shard_inputs(preds: np.ndarray, gts: np.ndarray, variant: str):
    """Build the interleaved low-precision layout
    X[b, tbp, p, i, g*128 + ii*8 + v] and slice per core (b outermost, so
    per-core slices are contiguous views)."""
    npdt = _np_dt(variant)
    p8 = np.asarray(preds).astype(npdt)
    g8 = np.asarray(gts).astype(npdt)
    X = np.empty((B, NTBP, 128, 2, 32, 16, NV), npdt)
    # preds [S, T, B, D] -> [b, tbp, p, i, g, ii, s]
    X[..., 0:S] = p8.reshape(S, 2, 2, 128, B, 32, 16).transpose(4, 1, 3, 2, 5, 6, 0)
    # gts [S, B, T, D] -> [b, tbp, p, i, g, ii, s]
    X[..., S : 2 * S] = g8.reshape(S, B, 2, 2, 128, 32, 16).transpose(
        1, 2, 4, 3, 5, 6, 0
    )
    X = X.reshape(B, NTBP, 128, 2, NCOL)
    in_maps = []
    for c in range(N_CORES):
        b0 = c * NB
        in_maps.append({"xa": X[b0 : b0 + NB], "id8": ID8, "id128": ID128})
    return in_maps


kernel_last_results = None


def kernel(preds: np.ndarray, gts: np.ndarray) -> np.ndarray:
    global kernel_last_results
    nc = _get_nc()
    in_maps = shard_inputs(preds, gts, VARIANT)
    trace = os.environ.get("MINLOSS_TRACE", "1") == "1"
    try:
        res = run_bass_kernel_spmd(
            nc, in_maps, core_ids=list(range(N_CORES)), trace=trace
        )
    except Exception:
        if not trace:
            raise
        # profiling infrastructure may be unavailable; rerun without it
        res = run_bass_kernel_spmd(
            nc, in_maps, core_ids=list(range(N_CORES)), trace=False
        )
    kernel_last_results = res
    total = 0.0
    for c in range(N_CORES):
        m2 = np.asarray(res.results[c]["loss"], dtype=np.float64)
        total += float(np.sqrt(np.maximum(m2, 0.0)).sum())
    return np.array(total, dtype=np.float32)


# revision 30
# speedup vs baseline: 1.0786x; 1.0052x over previous
"""Trainium2 Bass kernel for nn_MinLoss_12343736009330 (v2: fp8 DoubleRow).

Math: the reference loss is
    loss = sum_{b,s} || pf[b,s] - gf[b,match[b,s]] ||_2
where pf/gf are the per-(batch, source) flattened [L=T*D] signals, and match
is a greedy assignment on the 4x4 Euclidean cdist.  Since
    ||pf[s] - gf[m]||^2 = pn[s] + gn[m] - 2 <pf[s], gf[m]>,
the whole computation reduces to the per-batch 8x8 Gram matrix of the
8 vectors {pf[0..4], gf[0..4]} plus a tiny 4x4 greedy matching.

Key change vs v1: the host pre-casts the interleaved operand layout to
fp8 e4m3 (loss rel-err from input rounding ~4e-4, tolerance is 2e-2), which
cuts HBM traffic 4x vs fp32, and the Gram matmuls run in DoubleRow perf
mode (2 contraction rows per PE column-cycle).  DMA becomes the roofline:
4 MiB/core at ~332 GB/s ~= 12.6 us.

Sharding: batch axis (16) across 8 cores -> 2 batches/core.  Per batch the
t=512 contraction is covered by 2 tiles of [128 partitions, 2 (DoubleRow
halves), 4096 interleaved columns]; columns interleave (d-slice, vector) so
the 16 8x8 diagonal blocks of each accumulated 128x128 matmul hold per-
d-slice Gram contributions.  Selector matmuls reduce the diagonal blocks,
flatten matmuls put the Gram on one partition, and the greedy matching runs
on-device (min -> one-hot mask -> row/col conflict flags -> additive BIG
penalty).  The 8 greedy minima (squared) go back; host does sqrt + sum.
"""

import os
import sys

import ml_dtypes
import numpy as np

try:
    import concourse.bass as bass  # noqa: F401
except ImportError:
    sys.path.insert(0, "/opt/trn_rl_repo")

import concourse.bass as bass  # noqa: F811
import concourse.tile as tile
from concourse import bacc, mybir
from concourse.bass_utils import run_bass_kernel_spmd


def _install_ntff_hook_shim():
    """The bare agent image lacks ``antenv.axon_hooks``, so trace=True under
    axon would ImportError.  Recreate the module with the ctypes-based NTFF
    hook from trn_agent_boot (degrades to hook=None if unavailable)."""
    import types

    try:
        import antenv.axon_hooks  # noqa: F401

        return
    except ImportError:
        pass
    hook = None
    try:
        from trn_agent_boot.trn_boot import _ntff_profile_via_ctypes

        so_path = "/opt/axon/libaxon_pjrt.so"
        if os.path.exists(so_path):
            hook = _ntff_profile_via_ctypes(so_path)
    except Exception:
        hook = None
    import antenv

    mod = types.ModuleType("antenv.axon_hooks")
    mod.get_axon_ntff_profile_hook = lambda: hook  # type: ignore[attr-defined]

    def _set(h):
        nonlocal hook
        hook = h

    mod.set_axon_ntff_profile_hook = _set  # type: ignore[attr-defined]
    sys.modules["antenv.axon_hooks"] = mod
    antenv.axon_hooks = mod


_install_ntff_hook_shim()

F32 = mybir.dt.float32
BF16 = mybir.dt.bfloat16

S, T, B, D = 4, 512, 16, 512
N_CORES = 8
NB = B // N_CORES          # batches per core
NTBP = 2                   # t-block pairs per batch (4 blocks of 128 rows)
NV = 2 * S                 # 8 vectors per batch (4 preds + 4 gts)
NCOL = NV * D              # 4096 interleaved columns per DoubleRow half
NG = NCOL // 128           # 32 column groups per half
BIG = 1.0e30

# "fp8": e4m3 operands + DoubleRow matmuls (2 rows/cycle).
# "bf16": bf16 operands, plain matmuls.  Host pre-casts either way.
VARIANT = os.environ.get("MINLOSS_VARIANT", "fp8")
NSTRIP = int(os.environ.get("MINLOSS_NSTRIP", "4"))
# DMA issue queues, round-robin per strip: s=sync a=scalar v=vector g=gpsimd
QUEUES = os.environ.get("MINLOSS_QUEUES", "sa")

def _build_u():
    # U[v, 4s+m] = d(v,s) - d(v,4+m):  d2 = diag(U^T G U) = pn+gn-2*cross
    u = np.zeros((8, 16), np.float32)
    for s in range(4):
        for m in range(4):
            u[s, 4 * s + m] += 1.0
            u[4 + m, 4 * s + m] -= 1.0
    ubig = np.zeros((128, 256), np.float32)  # U at row-block q, col-block q
    for q in range(16):
        ubig[8 * q : 8 * q + 8, 16 * q : 16 * q + 16] = u
    return (
        ubig.astype(ml_dtypes.bfloat16),
        np.ascontiguousarray(u.T),          # [16, 8] f32
        np.eye(16, dtype=np.float32),
    )


UBIG, UTR, ID16 = _build_u()


def _bass_dt(variant: str):
    return mybir.dt.float8e4 if variant == "fp8" else BF16


def _np_dt(variant: str):
    return ml_dtypes.float8_e4m3 if variant == "fp8" else ml_dtypes.bfloat16


def build_nc(variant: str, nstrip: int, queues: str):
    nc = bacc.Bacc(
        "TRN2",
        target_bir_lowering=False,
        debug=False,
        enable_asserts=True,
        num_devices=N_CORES,
    )
    bdt = _bass_dt(variant)
    # xa[b, tbp, p, i, g*128 + ii*8 + v]: vector v's value at
    # t = 256*tbp + 128*i + p, d = 16*g + ii (v 0..3 preds, 4..7 gts).
    xa_t = nc.dram_tensor(
        "xa", [NB, NTBP, 128, 2, NCOL], bdt, kind="ExternalInput"
    ).ap()
    ubig_t = nc.dram_tensor("ubig", [128, 256], BF16, kind="ExternalInput").ap()
    utr_t = nc.dram_tensor("utr", [16, 8], F32, kind="ExternalInput").ap()
    id16_t = nc.dram_tensor("id16", [16, 16], F32, kind="ExternalInput").ap()
    # the 8 greedy minima (squared distances); host does sqrt + sum
    loss_t = nc.dram_tensor("loss", [1, 2 * S], F32, kind="ExternalOutput").ap()

    with tile.TileContext(nc) as tc:
        _build_tile(tc, xa_t, ubig_t, utr_t, id16_t, loss_t, variant, nstrip, queues)

    nc.compile()
    return nc


def _build_tile(tc, xa_t, ubig_t, utr_t, id16_t, loss_t, variant, nstrip, queues):
    nc = tc.nc
    import contextlib

    ctx = contextlib.ExitStack()
    with ctx:
        b_pool = ctx.enter_context(tc.tile_pool(name="b", bufs=NB * NTBP))
        psum_pool = ctx.enter_context(tc.tile_pool(name="psum", bufs=NB, space="PSUM"))
        psumf_pool = ctx.enter_context(tc.tile_pool(name="psumf", bufs=2, space="PSUM"))
        consts_pool = ctx.enter_context(tc.tile_pool(name="consts", bufs=1))
        small_pool = ctx.enter_context(tc.tile_pool(name="small", bufs=2))

        bdt = _bass_dt(variant)
        qmap = {"s": nc.sync, "a": nc.scalar, "v": nc.vector, "g": nc.gpsimd}
        qlist = [qmap[ch] for ch in queues]

        ubig = consts_pool.tile([128, 256], BF16, tag="ubig")
        utr = consts_pool.tile([16, 8], F32, tag="utr")
        id16 = consts_pool.tile([16, 16], F32, tag="id16")
        nc.gpsimd.dma_start(out=ubig[:, :], in_=ubig_t[:, :])
        nc.gpsimd.dma_start(out=utr[:, :], in_=utr_t[:, :])
        nc.gpsimd.dma_start(out=id16[:, :], in_=id16_t[:, :])

        # the 8 greedy minima (squared dists); sqrt+sum on host at the end
        loss4 = small_pool.tile([1, 2 * S], F32, tag="loss4")

        # ======== phase 0: issue ALL input strips up front ========
        # Strips round-robin over the issue queues so the DMA engines see a
        # continuous descriptor supply; tiles land roughly in order, matmuls
        # chase the strips.
        tiles = []
        qi = 0
        cw = NCOL // nstrip
        for ib in range(NB):
            for tbp in range(NTBP):
                btl = b_pool.tile([128, 2, NCOL], bdt, name=f"btl_{ib}_{tbp}")
                tiles.append(btl)
                for st in range(nstrip):
                    sl = slice(st * cw, (st + 1) * cw)
                    q = qlist[qi % len(qlist)]
                    qi += 1
                    q.dma_start(out=btl[:, :, sl], in_=xa_t[ib, tbp, :, :, sl])

        # ======== per batch: Gram matmuls + reduction + matching ========
        for ib in range(NB):
            psum = psum_pool.tile([128, 128], F32, name=f"psum_{ib}")
            for tbp in range(NTBP):
                btl = tiles[ib * NTBP + tbp]
                for g in range(NG):
                    first = tbp == 0 and g == 0
                    last = tbp == NTBP - 1 and g == NG - 1
                    if variant == "fp8":
                        op = btl[:, :, g * 128 : (g + 1) * 128]
                        nc.tensor.matmul(
                            psum[:, :],
                            lhsT=op,
                            rhs=op,
                            start=first,
                            stop=last,
                            perf_mode=mybir.MatmulPerfMode.DoubleRow,
                        )
                    else:
                        for i in range(2):
                            opi = btl[:, i, g * 128 : (g + 1) * 128]
                            nc.tensor.matmul(
                                psum[:, :],
                                lhsT=opi,
                                rhs=opi,
                                start=first and i == 0,
                                stop=last and i == 1,
                            )

            # ------------- diagonal-block reduction (on PE, U-fused) ------
            # Engine APs can't start at partition 8q, so selector matmuls
            # UBIG[:,16q:16q+16].T @ C[:,8q:8q+8] both move block q off
            # partition 8q and left-multiply by U, PSUM-accumulating
            # psu = U^T G over q.  d2 = diag(U^T G U) then comes from one
            # per-partition dot with U^T (DVE) plus a 1-row flatten matmul,
            # replacing the separate flatten + pn/gn/cross arithmetic.
            # bf16 copy halves DVE cost; partial-Gram bf16 rounding is
            # ~1e-4 on the loss.
            c_sb = small_pool.tile([128, 128], BF16, name=f"c_sb_{ib}")
            nc.vector.tensor_copy(out=c_sb[:, :], in_=psum[:, :])
            psu = psumf_pool.tile([16, 8], F32, tag="psu", name=f"psu_{ib}")
            for q in range(16):
                nc.tensor.matmul(
                    psu[:, :],
                    lhsT=ubig[:, 16 * q : 16 * q + 16],
                    rhs=c_sb[:, 8 * q : 8 * q + 8],
                    start=(q == 0),
                    stop=(q == 15),
                )
            sbu = small_pool.tile([16, 8], F32, name=f"sbu_{ib}")
            nc.vector.tensor_mul(out=sbu[:, :], in0=psu[:, :], in1=utr[:, :])
            d2p = small_pool.tile([16, 1], F32, name=f"d2p_{ib}")
            nc.vector.tensor_reduce(
                out=d2p[:, :],
                in_=sbu[:, :],
                axis=mybir.AxisListType.X,
                op=mybir.AluOpType.add,
            )
            psd2 = psumf_pool.tile([1, 16], F32, tag="psd2", name=f"psd2_{ib}")
            nc.tensor.matmul(
                psd2[:, :],
                lhsT=d2p[:, :],
                rhs=id16[:, :],
                start=True,
                stop=True,
            )

            # (sqrt is monotone, so the greedy matching runs on d2; the
            # sqrt of the 8 collected minima happens on the host)
            d2 = small_pool.tile([1, 16], F32, name=f"d2_{ib}")
            d2v = d2[0:1, :].rearrange("p (a b) -> p a b", b=4)
            nc.vector.tensor_copy(out=d2[:, :], in_=psd2[:, :])

            # ---------------- greedy matching on d2 ----------------
            # per iteration: min -> {0,BIG} mask of the argmin -> row/col
            # conflict flags (max-reduce over the 4x4 mask) -> add both
            # flags into d2.  (On an exact fp32 tie both tied entries are
            # masked; the resulting loss difference is O(tie gap).)
            mask16 = small_pool.tile([1, 16], F32, name=f"mask16_{ib}")
            rc = small_pool.tile([1, 8], F32, name=f"rc_{ib}")
            m44 = mask16[0:1, :].rearrange("p (r c) -> p r c", c=4)
            m44t = m44.transpose([0, 2, 1])
            rcv = rc[0:1, :].rearrange("p (x y) -> p x y", y=4)
            rowb = rcv[:, 0:1, :].transpose([0, 2, 1]).broadcast_to((1, 4, 4))
            colb = rcv[:, 1:2, :].broadcast_to((1, 4, 4))

            for it in range(S):
                slot = loss4[0:1, ib * S + it : ib * S + it + 1]
                nc.vector.tensor_reduce(
                    out=slot,
                    in_=d2[:, :],
                    axis=mybir.AxisListType.X,
                    op=mybir.AluOpType.min,
                )
                if it == S - 1:
                    break
                nc.vector.tensor_scalar(
                    out=mask16[:, :],
                    in0=d2[:, :],
                    scalar1=slot,
                    scalar2=BIG,
                    op0=mybir.AluOpType.is_le,
                    op1=mybir.AluOpType.mult,
                )
                nc.vector.tensor_reduce(
                    out=rc[0:1, 0:4],
                    in_=m44,
                    axis=mybir.AxisListType.X,
                    op=mybir.AluOpType.max,
                )
                nc.vector.tensor_reduce(
                    out=rc[0:1, 4:8],
                    in_=m44t,
                    axis=mybir.AxisListType.X,
                    op=mybir.AluOpType.max,
                )
                nc.vector.tensor_add(out=d2v, in0=d2v, in1=rowb)
                nc.vector.tensor_add(out=d2v, in0=d2v, in1=colb)

        nc.sync.dma_start(out=loss_t[0:1, :], in_=loss4[:, :])


_NC_CACHE: dict = {}


def _get_nc():
    key = (VARIANT, NSTRIP, QUEUES)
    if key not in _NC_CACHE:
        _NC_CACHE[key] = build_nc(*key)
    return _NC_CACHE[key]


def shard_inputs(preds: np.ndarray, gts: np.ndarray, variant: str):
    """Build the interleaved low-precision layout
    X[b, tbp, p, i, g*128 + ii*8 + v] and slice per core (b outermost, so
    per-core slices are contiguous views)."""
    npdt = _np_dt(variant)
    p8 = np.asarray(preds).astype(npdt)
    g8 = np.asarray(gts).astype(npdt)
    X = np.empty((B, NTBP, 128, 2, 32, 16, NV), npdt)
    # preds [S, T, B, D] -> [b, tbp, p, i, g, ii, s]
    X[..., 0:S] = p8.reshape(S, 2, 2, 128, B, 32, 16).transpose(4, 1, 3, 2, 5, 6, 0)
    # gts [S, B, T, D] -> [b, tbp, p, i, g, ii, s]
    X[..., S : 2 * S] = g8.reshape(S, B, 2, 2, 128, 32, 16).transpose(
        1, 2, 4, 3, 5, 6, 0
    )
    X = X.reshape(B, NTBP, 128, 2, NCOL)
    in_maps = []
    for c in range(N_CORES):
        b0 = c * NB
        in_maps.append(
            {"xa": X[b0 : b0 + NB], "ubig": UBIG, "utr": UTR, "id16": ID16}
        )
    return in_maps


kernel_last_results = None


def kernel(preds: np.ndarray, gts: np.ndarray) -> np.ndarray:
    global kernel_last_results
    nc = _get_nc()
    in_maps = shard_inputs(preds, gts, VARIANT)
    trace = os.environ.get("MINLOSS_TRACE", "1") == "1"
    try:
        res = run_bass_kernel_spmd(
            nc, in_maps, core_ids=list(range(N_CORES)), trace=trace
        )
    except Exception:
        if not trace:
            raise
        # profiling infrastructure may be unavailable; rerun without it
        res = run_bass_kernel_spmd(
            nc, in_maps, core_ids=list(range(N_CORES)), trace=False
        )
    kernel_last_results = res
    total = 0.0
    for c in range(N_CORES):
        m2 = np.asarray(res.results[c]["loss"], dtype=np.float64)
        total += float(np.sqrt(np.maximum(m2, 0.0)).sum())
    return np.array(total, dtype=np.float32)


# revision 31
# speedup vs baseline: 1.1291x; 1.0468x over previous
"""Trainium2 Bass kernel for nn_MinLoss_12343736009330 (v2: fp8 DoubleRow).

Math: the reference loss is
    loss = sum_{b,s} || pf[b,s] - gf[b,match[b,s]] ||_2
where pf/gf are the per-(batch, source) flattened [L=T*D] signals, and match
is a greedy assignment on the 4x4 Euclidean cdist.  Since
    ||pf[s] - gf[m]||^2 = pn[s] + gn[m] - 2 <pf[s], gf[m]>,
the whole computation reduces to the per-batch 8x8 Gram matrix of the
8 vectors {pf[0..4], gf[0..4]} plus a tiny 4x4 greedy matching.

Key change vs v1: the host pre-casts the interleaved operand layout to
fp8 e4m3 (loss rel-err from input rounding ~4e-4, tolerance is 2e-2), which
cuts HBM traffic 4x vs fp32, and the Gram matmuls run in DoubleRow perf
mode (2 contraction rows per PE column-cycle).  DMA becomes the roofline:
4 MiB/core at ~332 GB/s ~= 12.6 us.

Sharding: batch axis (16) across 8 cores -> 2 batches/core.  Per batch the
t=512 contraction is covered by 2 tiles of [128 partitions, 2 (DoubleRow
halves), 4096 interleaved columns]; columns interleave (d-slice, vector) so
the 16 8x8 diagonal blocks of each accumulated 128x128 matmul hold per-
d-slice Gram contributions.  U-fused selector matmuls reduce the diagonal
blocks while left-multiplying by U (u_j = e_s - e_{4+m}), so d2 =
diag(U^T G U) = pn + gn - 2*cross falls out of one per-partition dot plus a
1-row flatten matmul; the greedy matching runs on-device (min -> one-hot
mask -> row/col conflict flags -> additive BIG penalty).  The 8 greedy
minima (squared) go back; host does sqrt + sum.
"""

import os
import sys

import ml_dtypes
import numpy as np

try:
    import concourse.bass as bass  # noqa: F401
except ImportError:
    sys.path.insert(0, "/opt/trn_rl_repo")

import concourse.bass as bass  # noqa: F811
import concourse.tile as tile
from concourse import bacc, mybir
from concourse.bass_utils import run_bass_kernel_spmd


def _install_ntff_hook_shim():
    """The bare agent image lacks ``antenv.axon_hooks``, so trace=True under
    axon would ImportError.  Recreate the module with the ctypes-based NTFF
    hook from trn_agent_boot (degrades to hook=None if unavailable)."""
    import types

    try:
        import antenv.axon_hooks  # noqa: F401

        return
    except ImportError:
        pass
    hook = None
    try:
        from trn_agent_boot.trn_boot import _ntff_profile_via_ctypes

        so_path = "/opt/axon/libaxon_pjrt.so"
        if os.path.exists(so_path):
            hook = _ntff_profile_via_ctypes(so_path)
    except Exception:
        hook = None
    import antenv

    mod = types.ModuleType("antenv.axon_hooks")
    mod.get_axon_ntff_profile_hook = lambda: hook  # type: ignore[attr-defined]

    def _set(h):
        nonlocal hook
        hook = h

    mod.set_axon_ntff_profile_hook = _set  # type: ignore[attr-defined]
    sys.modules["antenv.axon_hooks"] = mod
    antenv.axon_hooks = mod


_install_ntff_hook_shim()

F32 = mybir.dt.float32
BF16 = mybir.dt.bfloat16

S, T, B, D = 4, 512, 16, 512
N_CORES = 8
NB = B // N_CORES          # batches per core
NTBP = 2                   # t-block pairs per batch (4 blocks of 128 rows)
NV = 2 * S                 # 8 vectors per batch (4 preds + 4 gts)
NCOL = NV * D              # 4096 interleaved columns per DoubleRow half
NG = NCOL // 128           # 32 column groups per half
BIG = 1.0e30

# "fp8": e4m3 operands + DoubleRow matmuls (2 rows/cycle).
# "bf16": bf16 operands, plain matmuls.  Host pre-casts either way.
VARIANT = os.environ.get("MINLOSS_VARIANT", "fp8")
NSTRIP = int(os.environ.get("MINLOSS_NSTRIP", "4"))
# DMA issue queues, round-robin per strip: s=sync a=scalar v=vector g=gpsimd
QUEUES = os.environ.get("MINLOSS_QUEUES", "sa")

def _build_u():
    # U[v, 4s+m] = d(v,s) - d(v,4+m):  d2 = diag(U^T G U) = pn+gn-2*cross
    u = np.zeros((8, 16), np.float32)
    for s in range(4):
        for m in range(4):
            u[s, 4 * s + m] += 1.0
            u[4 + m, 4 * s + m] -= 1.0
    ubig = np.zeros((128, 256), np.float32)  # U at row-block q, col-block q
    for q in range(16):
        ubig[8 * q : 8 * q + 8, 16 * q : 16 * q + 16] = u
    return (
        ubig.astype(ml_dtypes.bfloat16),
        np.ascontiguousarray(u.T),          # [16, 8] f32
        np.eye(16, dtype=np.float32),
    )


UBIG, UTR, ID16 = _build_u()


def _bass_dt(variant: str):
    return mybir.dt.float8e4 if variant == "fp8" else BF16


def _np_dt(variant: str):
    return ml_dtypes.float8_e4m3 if variant == "fp8" else ml_dtypes.bfloat16


def build_nc(variant: str, nstrip: int, queues: str):
    nc = bacc.Bacc(
        "TRN2",
        target_bir_lowering=False,
        debug=False,
        enable_asserts=True,
        num_devices=N_CORES,
    )
    bdt = _bass_dt(variant)
    # xa[b, tbp, p, i, g*128 + ii*8 + v]: vector v's value at
    # t = 256*tbp + 128*i + p, d = 16*g + ii (v 0..3 preds, 4..7 gts).
    xa_t = nc.dram_tensor(
        "xa", [NB, NTBP, 128, 2, NCOL], bdt, kind="ExternalInput"
    ).ap()
    ubig_t = nc.dram_tensor("ubig", [128, 256], BF16, kind="ExternalInput").ap()
    utr_t = nc.dram_tensor("utr", [16, 8], F32, kind="ExternalInput").ap()
    id16_t = nc.dram_tensor("id16", [16, 16], F32, kind="ExternalInput").ap()
    # the 8 greedy minima (squared distances); host does sqrt + sum
    loss_t = nc.dram_tensor("loss", [1, 2 * S], F32, kind="ExternalOutput").ap()

    with tile.TileContext(nc) as tc:
        _build_tile(tc, xa_t, ubig_t, utr_t, id16_t, loss_t, variant, nstrip, queues)

    nc.compile()
    return nc


def _build_tile(tc, xa_t, ubig_t, utr_t, id16_t, loss_t, variant, nstrip, queues):
    nc = tc.nc
    import contextlib

    ctx = contextlib.ExitStack()
    with ctx:
        b_pool = ctx.enter_context(tc.tile_pool(name="b", bufs=NB * NTBP))
        psum_pool = ctx.enter_context(tc.tile_pool(name="psum", bufs=NB, space="PSUM"))
        psumf_pool = ctx.enter_context(tc.tile_pool(name="psumf", bufs=2, space="PSUM"))
        consts_pool = ctx.enter_context(tc.tile_pool(name="consts", bufs=1))
        small_pool = ctx.enter_context(tc.tile_pool(name="small", bufs=2))

        bdt = _bass_dt(variant)
        qmap = {"s": nc.sync, "a": nc.scalar, "v": nc.vector, "g": nc.gpsimd}
        qlist = [qmap[ch] for ch in queues]

        ubig = consts_pool.tile([128, 256], BF16, tag="ubig")
        utr = consts_pool.tile([16, 8], F32, tag="utr")
        id16 = consts_pool.tile([16, 16], F32, tag="id16")
        nc.gpsimd.dma_start(out=ubig[:, :], in_=ubig_t[:, :])
        nc.gpsimd.dma_start(out=utr[:, :], in_=utr_t[:, :])
        nc.gpsimd.dma_start(out=id16[:, :], in_=id16_t[:, :])

        # the 8 greedy minima (squared dists); sqrt+sum on host at the end
        loss4 = small_pool.tile([1, 2 * S], F32, tag="loss4")

        # ======== phase 0: issue ALL input strips up front ========
        # Strips round-robin over the issue queues so the DMA engines see a
        # continuous descriptor supply; tiles land roughly in order, matmuls
        # chase the strips.
        tiles = []
        qi = 0
        cw = NCOL // nstrip
        for ib in range(NB):
            for tbp in range(NTBP):
                btl = b_pool.tile([128, 2, NCOL], bdt, name=f"btl_{ib}_{tbp}")
                tiles.append(btl)
                for st in range(nstrip):
                    sl = slice(st * cw, (st + 1) * cw)
                    q = qlist[qi % len(qlist)]
                    qi += 1
                    q.dma_start(out=btl[:, :, sl], in_=xa_t[ib, tbp, :, :, sl])

        # ======== per batch: Gram matmuls + reduction + matching ========
        for ib in range(NB):
            psum = psum_pool.tile([128, 128], F32, name=f"psum_{ib}")
            for tbp in range(NTBP):
                btl = tiles[ib * NTBP + tbp]
                for g in range(NG):
                    first = tbp == 0 and g == 0
                    last = tbp == NTBP - 1 and g == NG - 1
                    if variant == "fp8":
                        op = btl[:, :, g * 128 : (g + 1) * 128]
                        nc.tensor.matmul(
                            psum[:, :],
                            lhsT=op,
                            rhs=op,
                            start=first,
                            stop=last,
                            perf_mode=mybir.MatmulPerfMode.DoubleRow,
                        )
                    else:
                        for i in range(2):
                            opi = btl[:, i, g * 128 : (g + 1) * 128]
                            nc.tensor.matmul(
                                psum[:, :],
                                lhsT=opi,
                                rhs=opi,
                                start=first and i == 0,
                                stop=last and i == 1,
                            )

            # ------------- diagonal-block reduction (on PE, U-fused) ------
            # Engine APs can't start at partition 8q, so selector matmuls
            # UBIG[:,16q:16q+16].T @ C[:,8q:8q+8] both move block q off
            # partition 8q and left-multiply by U, PSUM-accumulating
            # psu = U^T G over q.  d2 = diag(U^T G U) then comes from one
            # per-partition dot with U^T (DVE) plus a 1-row flatten matmul,
            # replacing the separate flatten + pn/gn/cross arithmetic.
            # bf16 copy halves DVE cost; partial-Gram bf16 rounding is
            # ~1e-4 on the loss.
            c_sb = small_pool.tile([128, 128], BF16, name=f"c_sb_{ib}")
            nc.vector.tensor_copy(out=c_sb[:, :], in_=psum[:, :])
            psu = psumf_pool.tile([16, 8], F32, tag="psu", name=f"psu_{ib}")
            for q in range(16):
                nc.tensor.matmul(
                    psu[:, :],
                    lhsT=ubig[:, 16 * q : 16 * q + 16],
                    rhs=c_sb[:, 8 * q : 8 * q + 8],
                    start=(q == 0),
                    stop=(q == 15),
                )
            sbu = small_pool.tile([16, 8], F32, name=f"sbu_{ib}")
            nc.vector.tensor_mul(out=sbu[:, :], in0=psu[:, :], in1=utr[:, :])
            d2p = small_pool.tile([16, 1], F32, name=f"d2p_{ib}")
            nc.vector.tensor_reduce(
                out=d2p[:, :],
                in_=sbu[:, :],
                axis=mybir.AxisListType.X,
                op=mybir.AluOpType.add,
            )
            psd2 = psumf_pool.tile([1, 16], F32, tag="psd2", name=f"psd2_{ib}")
            nc.tensor.matmul(
                psd2[:, :],
                lhsT=d2p[:, :],
                rhs=id16[:, :],
                start=True,
                stop=True,
            )

            # (sqrt is monotone, so the greedy matching runs on d2; the
            # sqrt of the 8 collected minima happens on the host)
            d2 = small_pool.tile([1, 16], F32, name=f"d2_{ib}")
            d2v = d2[0:1, :].rearrange("p (a b) -> p a b", b=4)
            nc.vector.tensor_copy(out=d2[:, :], in_=psd2[:, :])

            # ---------------- greedy matching on d2 ----------------
            # per iteration: min -> {0,BIG} mask of the argmin -> row/col
            # conflict flags (max-reduce over the 4x4 mask) -> add both
            # flags into d2.  (On an exact fp32 tie both tied entries are
            # masked; the resulting loss difference is O(tie gap).)
            mask16 = small_pool.tile([1, 16], F32, name=f"mask16_{ib}")
            rc = small_pool.tile([1, 8], F32, name=f"rc_{ib}")
            m44 = mask16[0:1, :].rearrange("p (r c) -> p r c", c=4)
            m44t = m44.transpose([0, 2, 1])
            rcv = rc[0:1, :].rearrange("p (x y) -> p x y", y=4)
            rowb = rcv[:, 0:1, :].transpose([0, 2, 1]).broadcast_to((1, 4, 4))
            colb = rcv[:, 1:2, :].broadcast_to((1, 4, 4))

            for it in range(S):
                slot = loss4[0:1, ib * S + it : ib * S + it + 1]
                nc.vector.tensor_reduce(
                    out=slot,
                    in_=d2[:, :],
                    axis=mybir.AxisListType.X,
                    op=mybir.AluOpType.min,
                )
                if it == S - 1:
                    break
                nc.vector.tensor_scalar(
                    out=mask16[:, :],
                    in0=d2[:, :],
                    scalar1=slot,
                    scalar2=BIG,
                    op0=mybir.AluOpType.is_le,
                    op1=mybir.AluOpType.mult,
                )
                nc.vector.tensor_reduce(
                    out=rc[0:1, 0:4],
                    in_=m44,
                    axis=mybir.AxisListType.X,
                    op=mybir.AluOpType.max,
                )
                nc.vector.tensor_reduce(
                    out=rc[0:1, 4:8],
                    in_=m44t,
                    axis=mybir.AxisListType.X,
                    op=mybir.AluOpType.max,
                )
                nc.vector.tensor_add(out=d2v, in0=d2v, in1=rowb)
                nc.vector.tensor_add(out=d2v, in0=d2v, in1=colb)

        nc.sync.dma_start(out=loss_t[0:1, :], in_=loss4[:, :])


_NC_CACHE: dict = {}


def _get_nc():
    key = (VARIANT, NSTRIP, QUEUES)
    if key not in _NC_CACHE:
        _NC_CACHE[key] = build_nc(*key)
    return _NC_CACHE[key]


def shard_inputs(preds: np.ndarray, gts: np.ndarray, variant: str):
    """Build the interleaved low-precision layout
    X[b, tbp, p, i, g*128 + ii*8 + v] and slice per core (b outermost, so
    per-core slices are contiguous views)."""
    npdt = _np_dt(variant)
    p8 = np.asarray(preds).astype(npdt)
    g8 = np.asarray(gts).astype(npdt)
    X = np.empty((B, NTBP, 128, 2, 32, 16, NV), npdt)
    # preds [S, T, B, D] -> [b, tbp, p, i, g, ii, s]
    X[..., 0:S] = p8.reshape(S, 2, 2, 128, B, 32, 16).transpose(4, 1, 3, 2, 5, 6, 0)
    # gts [S, B, T, D] -> [b, tbp, p, i, g, ii, s]
    X[..., S : 2 * S] = g8.reshape(S, B, 2, 2, 128, 32, 16).transpose(
        1, 2, 4, 3, 5, 6, 0
    )
    X = X.reshape(B, NTBP, 128, 2, NCOL)
    in_maps = []
    for c in range(N_CORES):
        b0 = c * NB
        in_maps.append(
            {"xa": X[b0 : b0 + NB], "ubig": UBIG, "utr": UTR, "id16": ID16}
        )
    return in_maps


kernel_last_results = None


def kernel(preds: np.ndarray, gts: np.ndarray) -> np.ndarray:
    global kernel_last_results
    nc = _get_nc()
    in_maps = shard_inputs(preds, gts, VARIANT)
    trace = os.environ.get("MINLOSS_TRACE", "1") == "1"
    try:
        res = run_bass_kernel_spmd(
            nc, in_maps, core_ids=list(range(N_CORES)), trace=trace
        )
    except Exception:
        if not trace:
            raise
        # profiling infrastructure may be unavailable; rerun without it
        res = run_bass_kernel_spmd(
            nc, in_maps, core_ids=list(range(N_CORES)), trace=False
        )
    kernel_last_results = res
    total = 0.0
    for c in range(N_CORES):
        m2 = np.asarray(res.results[c]["loss"], dtype=np.float64)
        total += float(np.sqrt(np.maximum(m2, 0.0)).sum())
    return np.array(total, dtype=np.float32)
